# revision 24
# baseline (speedup 1.0000x reference)
"""MoE BERT block kernel for 8 Trainium2 NeuronCores.

Strategy: tensor-parallel over the expert FFN's INTER dimension. The router
(gate matmul + softmax + top-2) is a ~134 MFLOP computation done on the host
in float64 while packing the inputs; tokens are gathered into per-expert
segments on the host. Every core receives ALL 16384 token-expert pairs but
only a 512-wide slice of each expert's inter dimension:

    core c:  h_c   = gelu(Wup[e][c*512:(c+1)*512] @ x + bup_slice)   per token
             y_c   = Wdown[e][:, c*512:(c+1)*512] @ h_c              (partial)

gelu is elementwise in the inter dim, so y = sum_c y_c exactly. The host sums
the 8 f16 partials (float32 accumulate), adds bdown, and scatter-adds w * y.

Why this beats expert-parallel: per-core work is 64 matmul columns x 16384
tokens = 1.048M PE cycles regardless of the router outcome — perfect load
balance with zero token padding (expert-parallel pays for the heaviest
expert's 2161 tokens = 1.107M cycles). SBUF weight footprint is unchanged
(1/8 of every expert = 16.8 MB bf16).

DMA routing (descriptor issue costs ~0.6us of sequencer time per dma_start,
so the startup-critical stream must be few, large transfers in arrival
order): the sync (SP) ring carries tile 0's x per-ko chunks interleaved with
expert 0's up-weight io-chunks, expert 0's down weights, then the x stream
(prefetch distance 2) and per-tile y writebacks. The otherwise-idle GpSimd
software-DGE ring streams experts 1..7's weights as one ~1MB DMA per tensor,
far ahead of use. The scalar ring only runs GELUs.
"""

import os

os.environ.setdefault("MYCRO_LOCAL_CACHE", "1")

import numpy as np
import ml_dtypes

import concourse.bass as bass
import concourse.bacc as bacc
import concourse.mybir as mybir
import concourse.tile as tile
from concourse.bass_utils import run_bass_kernel_spmd

NUM_EXPERTS = 8
TOP_K = 2
H = 1024
I = 4096
P = 128
NCORES = 8
SLICE = I // NCORES  # 512 inter rows per core
KO = H // P  # 8 contraction tiles for the up matmul
IOL = SLICE // P  # 4 local inter tiles (psum partitions up / contraction down)
HO = H // P  # 8 output tiles for the down matmul
NMAX = 512  # max token tile (psum bank holds 512 f32)

BF16 = mybir.dt.bfloat16
F16 = mybir.dt.float16
F32 = mybir.dt.float32

_programs = {}  # schedule tuple -> compiled Bacc
last_results = None  # BassKernelResults of the most recent run (for profiling)


def _token_tiles(n):
    """Split n tokens into tiles: first tile NMAX, rest spread evenly in
    (128, 512]. The big first tile gives the DMA stream time to fill the
    pipeline before the next expert's weights are needed."""
    if n == 0:
        return []
    if n <= NMAX:
        return [n]
    k = -(-n // NMAX)  # ceil
    if n - NMAX <= (k - 1) * NMAX:
        rest = n - NMAX
        kk = k - 1
        base, rem = divmod(rest, kk)
        return [NMAX] + [base + 1] * rem + [base] * (kk - rem)
    base, rem = divmod(n, k)
    return [base + 1] * rem + [base] * (k - rem)


def _build_program(schedule):
    """schedule: tuple of (expert, ntok) tiles, concatenated token order."""
    TT = sum(nt for _, nt in schedule)
    nc = bacc.Bacc("TRN2", target_bir_lowering=False)

    # wup column order is io-major within an expert: [e][io][ko][128] so the
    # first io group's weights are one small contiguous chunk at startup.
    xt = nc.dram_tensor("xt", [P, KO * TT], BF16, kind="ExternalInput")
    wup = nc.dram_tensor("wup", [P, NUM_EXPERTS * IOL * KO * P], BF16, kind="ExternalInput")
    wdn = nc.dram_tensor("wdn", [P, NUM_EXPERTS * IOL * H], BF16, kind="ExternalInput")
    bup = nc.dram_tensor("bup", [P, NUM_EXPERTS * IOL], F32, kind="ExternalInput")
    yt = nc.dram_tensor("yt", [H, TT], F16, kind="ExternalOutput")

    experts_in_order = []
    for e, _ in schedule:
        if not experts_in_order or experts_in_order[-1] != e:
            experts_in_order.append(e)
    e0 = experts_in_order[0]

    with tile.TileContext(nc) as tc:
        with (
            tc.tile_pool(name="weights", bufs=1) as wpool,
            tc.tile_pool(name="xin", bufs=5) as xpool,
            tc.tile_pool(name="hmid", bufs=2) as hpool,
            tc.tile_pool(name="yout", bufs=3) as ypool,
            tc.tile_pool(name="psum_up", bufs=4, space="PSUM") as pu,
            tc.tile_pool(name="psum_dn", bufs=4, space="PSUM") as pd,
        ):
            yt_r = yt.ap().rearrange("(ho p) t -> p ho t", p=P)
            xt_ap = xt.ap()
            wup_ap = wup.ap()
            wdn_ap = wdn.ap()

            wup_sb = wpool.tile([P, NUM_EXPERTS, IOL * KO * P], BF16, tag="wup")
            wdn_sb = wpool.tile([P, NUM_EXPERTS, IOL * H], BF16, tag="wdn")
            bup_sb = wpool.tile([P, NUM_EXPERTS * IOL], F32, tag="bup")

            # Warmup operand tile. The warmup matmuls accumulate into a
            # throwaway psum group that is never read (its bank is later
            # reset by a real start=True group), so the values don't matter;
            # the memset only exists because the framework requires every
            # read tile to have a writer.
            xw_sb = wpool.tile([P, NMAX], BF16, tag="warmx")
            nc.vector.memset(xw_sb[:], 0.0)

            # --- Startup stream on the sync ring, in arrival-need order:
            # bup (tiny, needed by the first GELU), then tile 0's x per-ko
            # chunks interleaved with expert 0's up-weight io-chunks (the
            # first real matmul needs only x0[ko0] + wup[e0,io0]), then
            # expert 0's down weights, then x1/x2 (prefetch distance 2).
            _, n0 = schedule[0]
            x0_sb = xpool.tile([P, KO, NMAX], BF16, tag="x")
            x0_r = xt_ap[:, 0 : KO * n0].rearrange("p (ko t) -> p ko t", ko=KO)
            for ko in range(KO):
                nc.sync.dma_start(x0_sb[:, ko, :n0], x0_r[:, ko])
                if ko < IOL:
                    io = ko
                    col = (e0 * IOL + io) * KO * P
                    nc.sync.dma_start(
                        wup_sb[:, e0, io * KO * P : (io + 1) * KO * P],
                        wup_ap[:, col : col + KO * P],
                    )
            nc.sync.dma_start(bup_sb[:], bup.ap())
            nc.sync.dma_start(
                wdn_sb[:, e0], wdn_ap[:, e0 * IOL * H : (e0 + 1) * IOL * H]
            )

            # Experts 1..7 weights: one ~1MB DMA per tensor on the sync ring.
            # The tile scheduler orders each queue by dependency readiness,
            # so dep-free weight DMAs would all hoist into the startup window
            # and starve tile 0's critical transfers (that cost v2 ~35us of
            # PE gaps). tile_wait_until stamps them with staggered modeled
            # times instead: each expert streams during the previous expert's
            # compute, ~3 tiles before first use.
            for k, en in enumerate(experts_in_order[1:]):
                with tc.tile_wait_until(0.015 + 0.045 * k):
                    nc.sync.dma_start(
                        wup_sb[:, en, :],
                        wup_ap[:, en * IOL * KO * P : (en + 1) * IOL * KO * P],
                    )
                    nc.sync.dma_start(
                        wdn_sb[:, en, :],
                        wdn_ap[:, en * IOL * H : (en + 1) * IOL * H],
                    )

            # Warmup chain: 12 garbage matmuls bridge the PE from engine-up
            # to tile 0's operand arrival; 6 more keep-alives are interleaved
            # between tile 0 io0's arrival-paced ko steps (below).
            NWARM, NKEEP = 6, 6
            ps_trash = pd.tile([P, NMAX], F32, tag="pd", name="trash")
            warm_i = [0]

            def warm_mm():
                nc.tensor.matmul(
                    ps_trash[:, :NMAX], lhsT=xw_sb[:, :P], rhs=xw_sb[:, :NMAX],
                    start=(warm_i[0] == 0),
                    stop=(warm_i[0] == NWARM + NKEEP - 1),
                )
                warm_i[0] += 1

            for _ in range(NWARM):
                warm_mm()

            offsets = []
            o = 0
            for _, nt in schedule:
                offsets.append(o)
                o += nt

            def prefetch_x(tp):
                npre = schedule[tp][1]
                opre = offsets[tp]
                xp = xpool.tile([P, KO, NMAX], BF16, tag="x")
                nc.sync.dma_start(
                    xp[:, :, :npre],
                    xt_ap[:, KO * opre : KO * (opre + npre)].rearrange(
                        "p (ko t) -> p ko t", ko=KO
                    ),
                )
                return xp

            XPRE = 4  # x prefetch distance (xpool bufs - 1)
            x_tiles = {0: x0_sb}
            for tp in range(1, XPRE + 1):
                if tp < len(schedule):
                    x_tiles[tp] = prefetch_x(tp)

            off = 0
            for t, (e, ntok) in enumerate(schedule):
                x_sb = x_tiles.pop(t)
                if t >= 1 and t + XPRE < len(schedule):
                    x_tiles[t + XPRE] = prefetch_x(t + XPRE)

                # --- Up-projection + exact (erf) GELU: h tile [512, ntok].
                # io-major (contraction inner): each group's GELU drains while
                # the next group accumulates — no end-of-tile burst.
                h_sb = hpool.tile([P, IOL, NMAX], BF16, tag="h")
                for io in range(IOL):
                    ps = pu.tile([P, NMAX], F32, tag="pu", name=f"pu{io}")
                    for ko in range(KO):
                        col = io * KO * P + ko * P
                        nc.tensor.matmul(
                            ps[:, :ntok],
                            lhsT=wup_sb[:, e, col : col + P],
                            rhs=x_sb[:, ko, :ntok],
                            start=(ko == 0),
                            stop=(ko == KO - 1),
                        )
                        if t == 0 and io == 0 and ko < NKEEP:
                            # Keep-alive between arrival-paced first ko steps.
                            warm_mm()
                    nc.scalar.activation(
                        h_sb[:, io, :ntok], ps[:, :ntok],
                        mybir.ActivationFunctionType.Gelu,
                        bias=bup_sb[:, e * IOL + io : e * IOL + io + 1],
                        scale=1.0,
                    )
                # --- Down-projection partial: y tile [1024, ntok] f16.
                # ho-major, contraction (4 io steps) inner. The psum->f16 cast
                # (DVE) overlaps the next ho's matmuls; y leaves in two DMAs
                # (ho0..6 mid-tile, ho7 at the end) to keep the sync ring at
                # two descriptors per tile.
                y_sb = ypool.tile([P, HO, NMAX], F16, tag="y")
                for ho in range(HO):
                    ps = pd.tile([P, NMAX], F32, tag="pd", name=f"pd{ho % 4}")
                    for io in range(IOL):
                        col = io * H + ho * P
                        nc.tensor.matmul(
                            ps[:, :ntok],
                            lhsT=wdn_sb[:, e, col : col + P],
                            rhs=h_sb[:, io, :ntok],
                            start=(io == 0),
                            stop=(io == IOL - 1),
                        )
                    nc.vector.tensor_scalar_add(
                        y_sb[:, ho, :ntok], ps[:, :ntok], 0.0
                    )
                    if ho == HO - 2:
                        nc.sync.dma_start(
                            yt_r[:, 0 : HO - 1, off : off + ntok],
                            y_sb[:, 0 : HO - 1, :ntok],
                        )
                nc.sync.dma_start(
                    yt_r[:, HO - 1, off : off + ntok], y_sb[:, HO - 1, :ntok]
                )
                off += ntok

    nc.compile()
    return nc


def _get_program(schedule):
    key = tuple(schedule)
    if key not in _programs:
        _programs[key] = _build_program(key)
    return _programs[key]


def _route(X64, Wg64):
    """Replicates the reference router: softmax over gate logits, top-2."""
    T = X64.shape[0]
    logits = X64 @ Wg64.T  # [T, E]
    logits -= logits.max(axis=-1, keepdims=True)
    p = np.exp(logits)
    p /= p.sum(axis=-1, keepdims=True)
    i1 = np.argmax(p, axis=-1)
    rows = np.arange(T)
    w1 = p[rows, i1]
    p2 = p.copy()
    p2[rows, i1] = -1.0
    i2 = np.argmax(p2, axis=-1)
    w2 = p[rows, i2]
    return i1, w1, i2, w2


def _pack_core(c, Wup16, Wdn16, bup):
    rows = slice(c * SLICE, (c + 1) * SLICE)
    wup_c = np.empty((P, NUM_EXPERTS * IOL * KO * P), dtype=ml_dtypes.bfloat16)
    wdn_c = np.empty((P, NUM_EXPERTS * IOL * H), dtype=ml_dtypes.bfloat16)
    bup_c = np.empty((P, NUM_EXPERTS * IOL), dtype=np.float32)
    for e in range(NUM_EXPERTS):
        # up lhsT: [k partition, (io, ko, 128 io-rows)] io-major chunks
        w = Wup16[e][rows, :].T.reshape(KO, P, IOL, P)  # [ko, kp, io, m]
        w = w.transpose(1, 2, 0, 3)  # [kp, io, ko, m]
        wup_c[:, e * IOL * KO * P : (e + 1) * IOL * KO * P] = w.reshape(P, -1)
        # down lhsT: [local-inter partition, (io, H cols)]
        d = Wdn16[e][:, rows].T.reshape(IOL, P, H).transpose(1, 0, 2)
        wdn_c[:, e * IOL * H : (e + 1) * IOL * H] = d.reshape(P, -1)
        bup_c[:, e * IOL : (e + 1) * IOL] = bup[e][rows].reshape(IOL, P).T
    return (
        np.ascontiguousarray(wup_c),
        np.ascontiguousarray(wdn_c),
        bup_c,
    )


def kernel(hidden_states, Wg, Wup, bup, Wdown, bdown):
    global last_results
    hidden_states = np.asarray(hidden_states)
    orig_shape = hidden_states.shape
    X = np.ascontiguousarray(hidden_states, dtype=np.float32).reshape(-1, H)
    T = X.shape[0]
    Wg = np.asarray(Wg, dtype=np.float32)
    Wup = np.asarray(Wup, dtype=np.float32)
    bup = np.asarray(bup, dtype=np.float32)
    Wdown = np.asarray(Wdown, dtype=np.float32)
    bdown = np.asarray(bdown, dtype=np.float32)

    # --- Router on host (float64 for a faithful top-2 ordering) ---
    i1, w1, i2, w2 = _route(X.astype(np.float64), Wg.astype(np.float64))

    # --- Dispatch: gather tokens into per-expert segments (i1 then i2) ---
    seg_idx, seg_wts = [], []
    schedule = []
    for e in range(NUM_EXPERTS):
        sel1 = np.nonzero(i1 == e)[0]
        sel2 = np.nonzero(i2 == e)[0]
        idx = np.concatenate([sel1, sel2])
        wts = np.concatenate([w1[sel1], w2[sel2]])
        seg_idx.append(idx)
        seg_wts.append(wts)
        schedule.extend((e, nt) for nt in _token_tiles(idx.size))
    schedule = tuple(schedule)
    idx_all = np.concatenate(seg_idx)

    # --- Pack device inputs ---
    Xb = X.astype(ml_dtypes.bfloat16)
    Xsel = Xb[idx_all]  # [TT, H]
    blocks = []
    o = 0
    for _, nt in schedule:
        blk = Xsel[o : o + nt].T.reshape(KO, P, nt)  # [KO, P, nt]
        blocks.append(blk.transpose(1, 0, 2).reshape(P, -1))
        o += nt
    xt_dev = np.ascontiguousarray(np.concatenate(blocks, axis=1))

    Wup16 = Wup.astype(ml_dtypes.bfloat16)
    Wdn16 = Wdown.astype(ml_dtypes.bfloat16)
    in_maps = []
    for c in range(NCORES):
        wup_c, wdn_c, bup_c = _pack_core(c, Wup16, Wdn16, bup)
        in_maps.append({"xt": xt_dev, "wup": wup_c, "wdn": wdn_c, "bup": bup_c})

    # --- Run the Bass kernel on all 8 cores ---
    nc = _get_program(schedule)
    last_results = run_bass_kernel_spmd(nc, in_maps, core_ids=list(range(NCORES)))

    # --- Combine: sum the 8 inter-slice partials, add bdown, scatter w * y ---
    ysum = np.zeros((H, sum(nt for _, nt in schedule)), dtype=np.float32)
    for c in range(NCORES):
        ysum += np.asarray(last_results.results[c]["yt"]).astype(np.float32)

    out = np.zeros((T, H), dtype=np.float32)
    o = 0
    for e in range(NUM_EXPERTS):
        n = seg_idx[e].size
        if n == 0:
            continue
        Y = ysum[:, o : o + n].T + bdown[e]
        out[seg_idx[e]] += seg_wts[e][:, None].astype(np.float32) * Y
        o += n
    return out.reshape(orig_shape)


# revision 26
# speedup vs baseline: 1.2363x; 1.2363x over previous
"""MoE BERT block kernel for 8 Trainium2 NeuronCores.

Strategy: tensor-parallel over the expert FFN's INTER dimension. The router
(gate matmul + softmax + top-2) is a ~134 MFLOP computation done on the host
in float64 while packing the inputs; tokens are gathered into per-expert
segments on the host. Every core receives ALL 16384 token-expert pairs but
only a 512-wide slice of each expert's inter dimension:

    core c:  h_c   = gelu(Wup[e][c*512:(c+1)*512] @ x + bup_slice)   per token
             y_c   = Wdown[e][:, c*512:(c+1)*512] @ h_c              (partial)

gelu is elementwise in the inter dim, so y = sum_c y_c exactly. The host sums
the 8 f16 partials (float32 accumulate), adds bdown, and scatter-adds w * y.

Why this beats expert-parallel: per-core work is 64 matmul columns x 16384
tokens = 1.048M PE cycles regardless of the router outcome — perfect load
balance with zero token padding (expert-parallel pays for the heaviest
expert's 2161 tokens = 1.107M cycles). SBUF weight footprint is unchanged
(1/8 of every expert = 16.8 MB bf16).

DMA routing (descriptor issue costs ~0.6us of sequencer time per dma_start,
so the startup-critical stream must be few, large transfers in arrival
order): the sync (SP) ring carries tile 0's x per-ko chunks interleaved with
expert 0's up-weight io-chunks, expert 0's down weights, then the x stream
(prefetch distance 2) and per-tile y writebacks. The otherwise-idle GpSimd
software-DGE ring streams experts 1..7's weights as one ~1MB DMA per tensor,
far ahead of use. The scalar ring only runs GELUs.
"""

import os

os.environ.setdefault("MYCRO_LOCAL_CACHE", "1")

import numpy as np
import ml_dtypes

import concourse.bass as bass
import concourse.bacc as bacc
import concourse.mybir as mybir
import concourse.tile as tile
from concourse.bass_utils import run_bass_kernel_spmd

NUM_EXPERTS = 8
TOP_K = 2
H = 1024
I = 4096
P = 128
NCORES = 8
SLICE = I // NCORES  # 512 inter rows per core
KO = H // P  # 8 contraction tiles for the up matmul
IOL = SLICE // P  # 4 local inter tiles (psum partitions up / contraction down)
HO = H // P  # 8 output tiles for the down matmul
NMAX = 512  # max token tile (psum bank holds 512 f32)

BF16 = mybir.dt.bfloat16
F16 = mybir.dt.float16
F32 = mybir.dt.float32

_programs = {}  # schedule tuple -> compiled Bacc
last_results = None  # BassKernelResults of the most recent run (for profiling)


def _token_tiles(n):
    """Split n tokens into tiles: first tile NMAX, rest spread evenly in
    (128, 512]. The big first tile gives the DMA stream time to fill the
    pipeline before the next expert's weights are needed."""
    if n == 0:
        return []
    if n <= NMAX:
        return [n]
    k = -(-n // NMAX)  # ceil
    if n - NMAX <= (k - 1) * NMAX:
        rest = n - NMAX
        kk = k - 1
        base, rem = divmod(rest, kk)
        return [NMAX] + [base + 1] * rem + [base] * (kk - rem)
    base, rem = divmod(n, k)
    return [base + 1] * rem + [base] * (k - rem)


def _build_program(schedule):
    """schedule: tuple of (expert, ntok) tiles, concatenated token order."""
    TT = sum(nt for _, nt in schedule)
    nc = bacc.Bacc("TRN2", target_bir_lowering=False)

    # wup column order is io-major within an expert: [e][io][ko][128] so the
    # first io group's weights are one small contiguous chunk at startup.
    xt = nc.dram_tensor("xt", [P, KO * TT], BF16, kind="ExternalInput")
    wup = nc.dram_tensor("wup", [P, NUM_EXPERTS * IOL * KO * P], BF16, kind="ExternalInput")
    wdn = nc.dram_tensor("wdn", [P, NUM_EXPERTS * IOL * H], BF16, kind="ExternalInput")
    bup = nc.dram_tensor("bup", [P, NUM_EXPERTS * IOL], F32, kind="ExternalInput")
    yt = nc.dram_tensor("yt", [H, TT], F16, kind="ExternalOutput")

    experts_in_order = []
    for e, _ in schedule:
        if not experts_in_order or experts_in_order[-1] != e:
            experts_in_order.append(e)
    e0 = experts_in_order[0]

    with tile.TileContext(nc) as tc:
        with (
            tc.tile_pool(name="weights", bufs=1) as wpool,
            tc.tile_pool(name="xin", bufs=5) as xpool,
            tc.tile_pool(name="hmid", bufs=2) as hpool,
            tc.tile_pool(name="yout", bufs=12) as ypool,
            tc.tile_pool(name="psum_up", bufs=3, space="PSUM") as pu,
            tc.tile_pool(name="psum_dn", bufs=5, space="PSUM") as pd,
        ):
            yt_r = yt.ap().rearrange("(ho p) t -> p ho t", p=P)
            xt_ap = xt.ap()
            wup_ap = wup.ap()
            wdn_ap = wdn.ap()

            wup_sb = wpool.tile([P, NUM_EXPERTS, IOL * KO * P], BF16, tag="wup")
            wdn_sb = wpool.tile([P, NUM_EXPERTS, IOL * H], BF16, tag="wdn")
            bup_sb = wpool.tile([P, NUM_EXPERTS * IOL], F32, tag="bup")

            # Warmup operand tile. The warmup matmuls accumulate into a
            # throwaway psum group that is never read (its bank is later
            # reset by a real start=True group), so the values don't matter;
            # the memset only exists because the framework requires every
            # read tile to have a writer.
            xw_sb = wpool.tile([P, NMAX], BF16, tag="warmx")
            nc.vector.memset(xw_sb[:], 0.0)

            # --- Startup stream on the sync ring, in arrival-need order:
            # bup (tiny, needed by the first GELU), then tile 0's x per-ko
            # chunks interleaved with expert 0's up-weight io-chunks (the
            # first real matmul needs only x0[ko0] + wup[e0,io0]), then
            # expert 0's down weights, then x1/x2 (prefetch distance 2).
            _, n0 = schedule[0]
            x0_sb = xpool.tile([P, KO, NMAX], BF16, tag="x")
            x0_r = xt_ap[:, 0 : KO * n0].rearrange("p (ko t) -> p ko t", ko=KO)
            for ko in range(KO):
                nc.sync.dma_start(x0_sb[:, ko, :n0], x0_r[:, ko])
                if ko < IOL:
                    io = ko
                    col = (e0 * IOL + io) * KO * P
                    nc.sync.dma_start(
                        wup_sb[:, e0, io * KO * P : (io + 1) * KO * P],
                        wup_ap[:, col : col + KO * P],
                    )
            nc.sync.dma_start(bup_sb[:], bup.ap())
            nc.sync.dma_start(
                wdn_sb[:, e0], wdn_ap[:, e0 * IOL * H : (e0 + 1) * IOL * H]
            )

            # Experts 1..7 weights: one ~1MB DMA per tensor on the sync ring.
            # The tile scheduler orders each queue by dependency readiness,
            # so dep-free weight DMAs would all hoist into the startup window
            # and starve tile 0's critical transfers (that cost v2 ~35us of
            # PE gaps). tile_wait_until stamps them with staggered modeled
            # times instead: each expert streams during the previous expert's
            # compute, ~3 tiles before first use.
            for k, en in enumerate(experts_in_order[1:]):
                with tc.tile_wait_until(0.015 + 0.045 * k):
                    nc.sync.dma_start(
                        wup_sb[:, en, :],
                        wup_ap[:, en * IOL * KO * P : (en + 1) * IOL * KO * P],
                    )
                    nc.sync.dma_start(
                        wdn_sb[:, en, :],
                        wdn_ap[:, en * IOL * H : (en + 1) * IOL * H],
                    )

            # Warmup chain: 12 garbage matmuls bridge the PE from engine-up
            # to tile 0's operand arrival; 6 more keep-alives are interleaved
            # between tile 0 io0's arrival-paced ko steps (below).
            NWARM, NKEEP = 6, 6
            ps_trash = pd.tile([P, NMAX], F32, tag="pd", name="trash")
            warm_i = [0]

            def warm_mm():
                nc.tensor.matmul(
                    ps_trash[:, :NMAX], lhsT=xw_sb[:, :P], rhs=xw_sb[:, :NMAX],
                    start=(warm_i[0] == 0),
                    stop=(warm_i[0] == NWARM + NKEEP - 1),
                )
                warm_i[0] += 1

            for _ in range(NWARM):
                warm_mm()

            offsets = []
            o = 0
            for _, nt in schedule:
                offsets.append(o)
                o += nt

            def prefetch_x(tp):
                npre = schedule[tp][1]
                opre = offsets[tp]
                xp = xpool.tile([P, KO, NMAX], BF16, tag="x")
                nc.sync.dma_start(
                    xp[:, :, :npre],
                    xt_ap[:, KO * opre : KO * (opre + npre)].rearrange(
                        "p (ko t) -> p ko t", ko=KO
                    ),
                )
                return xp

            XPRE = 4  # x prefetch distance (xpool bufs - 1)
            x_tiles = {0: x0_sb}
            for tp in range(1, XPRE + 1):
                if tp < len(schedule):
                    x_tiles[tp] = prefetch_x(tp)

            off = 0
            for t, (e, ntok) in enumerate(schedule):
                x_sb = x_tiles.pop(t)
                if t >= 1 and t + XPRE < len(schedule):
                    x_tiles[t + XPRE] = prefetch_x(t + XPRE)

                # --- Up-projection + exact (erf) GELU: h tile [512, ntok].
                # io-major (contraction inner): each group's GELU drains while
                # the next group accumulates — no end-of-tile burst.
                h_sb = hpool.tile([P, IOL, NMAX], BF16, tag="h")
                for io in range(IOL):
                    ps = pu.tile([P, NMAX], F32, tag="pu", name=f"pu{io}")
                    for ko in range(KO):
                        col = io * KO * P + ko * P
                        nc.tensor.matmul(
                            ps[:, :ntok],
                            lhsT=wup_sb[:, e, col : col + P],
                            rhs=x_sb[:, ko, :ntok],
                            start=(ko == 0),
                            stop=(ko == KO - 1),
                        )
                        if t == 0 and io == 0 and ko < NKEEP:
                            # Keep-alive between arrival-paced first ko steps.
                            warm_mm()
                    nc.scalar.activation(
                        h_sb[:, io, :ntok], ps[:, :ntok],
                        mybir.ActivationFunctionType.Gelu,
                        bias=bup_sb[:, e * IOL + io : e * IOL + io + 1],
                        scale=1.0,
                    )
                # --- Down-projection partial: y tile [1024, ntok] f16.
                # ho-major, contraction (4 io steps) inner. The psum->f16 cast
                # (DVE) overlaps the next ho's matmuls. y buffers and DMAs are
                # per-ho: under an HBM-bandwidth drought (cotenant noise) the
                # 12-slot rotation gives ~1.5 tiles of writeback slack before
                # the psum-bank chain would stall the PE.
                for ho in range(HO):
                    ps = pd.tile([P, NMAX], F32, tag="pd", name=f"pd{ho % 4}")
                    for io in range(IOL):
                        col = io * H + ho * P
                        nc.tensor.matmul(
                            ps[:, :ntok],
                            lhsT=wdn_sb[:, e, col : col + P],
                            rhs=h_sb[:, io, :ntok],
                            start=(io == 0),
                            stop=(io == IOL - 1),
                        )
                    y_sb = ypool.tile([P, NMAX], F16, tag="y")
                    nc.vector.tensor_scalar_add(
                        y_sb[:, :ntok], ps[:, :ntok], 0.0
                    )
                    nc.sync.dma_start(
                        yt_r[:, ho, off : off + ntok], y_sb[:, :ntok]
                    )
                off += ntok

    nc.compile()
    return nc


def _get_program(schedule):
    key = tuple(schedule)
    if key not in _programs:
        _programs[key] = _build_program(key)
    return _programs[key]


def _route(X64, Wg64):
    """Replicates the reference router: softmax over gate logits, top-2."""
    T = X64.shape[0]
    logits = X64 @ Wg64.T  # [T, E]
    logits -= logits.max(axis=-1, keepdims=True)
    p = np.exp(logits)
    p /= p.sum(axis=-1, keepdims=True)
    i1 = np.argmax(p, axis=-1)
    rows = np.arange(T)
    w1 = p[rows, i1]
    p2 = p.copy()
    p2[rows, i1] = -1.0
    i2 = np.argmax(p2, axis=-1)
    w2 = p[rows, i2]
    return i1, w1, i2, w2


def _pack_core(c, Wup16, Wdn16, bup):
    rows = slice(c * SLICE, (c + 1) * SLICE)
    wup_c = np.empty((P, NUM_EXPERTS * IOL * KO * P), dtype=ml_dtypes.bfloat16)
    wdn_c = np.empty((P, NUM_EXPERTS * IOL * H), dtype=ml_dtypes.bfloat16)
    bup_c = np.empty((P, NUM_EXPERTS * IOL), dtype=np.float32)
    for e in range(NUM_EXPERTS):
        # up lhsT: [k partition, (io, ko, 128 io-rows)] io-major chunks
        w = Wup16[e][rows, :].T.reshape(KO, P, IOL, P)  # [ko, kp, io, m]
        w = w.transpose(1, 2, 0, 3)  # [kp, io, ko, m]
        wup_c[:, e * IOL * KO * P : (e + 1) * IOL * KO * P] = w.reshape(P, -1)
        # down lhsT: [local-inter partition, (io, H cols)]
        d = Wdn16[e][:, rows].T.reshape(IOL, P, H).transpose(1, 0, 2)
        wdn_c[:, e * IOL * H : (e + 1) * IOL * H] = d.reshape(P, -1)
        bup_c[:, e * IOL : (e + 1) * IOL] = bup[e][rows].reshape(IOL, P).T
    return (
        np.ascontiguousarray(wup_c),
        np.ascontiguousarray(wdn_c),
        bup_c,
    )


def kernel(hidden_states, Wg, Wup, bup, Wdown, bdown):
    global last_results
    hidden_states = np.asarray(hidden_states)
    orig_shape = hidden_states.shape
    X = np.ascontiguousarray(hidden_states, dtype=np.float32).reshape(-1, H)
    T = X.shape[0]
    Wg = np.asarray(Wg, dtype=np.float32)
    Wup = np.asarray(Wup, dtype=np.float32)
    bup = np.asarray(bup, dtype=np.float32)
    Wdown = np.asarray(Wdown, dtype=np.float32)
    bdown = np.asarray(bdown, dtype=np.float32)

    # --- Router on host (float64 for a faithful top-2 ordering) ---
    i1, w1, i2, w2 = _route(X.astype(np.float64), Wg.astype(np.float64))

    # --- Dispatch: gather tokens into per-expert segments (i1 then i2) ---
    seg_idx, seg_wts = [], []
    schedule = []
    for e in range(NUM_EXPERTS):
        sel1 = np.nonzero(i1 == e)[0]
        sel2 = np.nonzero(i2 == e)[0]
        idx = np.concatenate([sel1, sel2])
        wts = np.concatenate([w1[sel1], w2[sel2]])
        seg_idx.append(idx)
        seg_wts.append(wts)
        schedule.extend((e, nt) for nt in _token_tiles(idx.size))
    schedule = tuple(schedule)
    idx_all = np.concatenate(seg_idx)

    # --- Pack device inputs ---
    Xb = X.astype(ml_dtypes.bfloat16)
    Xsel = Xb[idx_all]  # [TT, H]
    blocks = []
    o = 0
    for _, nt in schedule:
        blk = Xsel[o : o + nt].T.reshape(KO, P, nt)  # [KO, P, nt]
        blocks.append(blk.transpose(1, 0, 2).reshape(P, -1))
        o += nt
    xt_dev = np.ascontiguousarray(np.concatenate(blocks, axis=1))

    Wup16 = Wup.astype(ml_dtypes.bfloat16)
    Wdn16 = Wdown.astype(ml_dtypes.bfloat16)
    in_maps = []
    for c in range(NCORES):
        wup_c, wdn_c, bup_c = _pack_core(c, Wup16, Wdn16, bup)
        in_maps.append({"xt": xt_dev, "wup": wup_c, "wdn": wdn_c, "bup": bup_c})

    # --- Run the Bass kernel on all 8 cores ---
    nc = _get_program(schedule)
    last_results = run_bass_kernel_spmd(nc, in_maps, core_ids=list(range(NCORES)))

    # --- Combine: sum the 8 inter-slice partials, add bdown, scatter w * y ---
    ysum = np.zeros((H, sum(nt for _, nt in schedule)), dtype=np.float32)
    for c in range(NCORES):
        ysum += np.asarray(last_results.results[c]["yt"]).astype(np.float32)

    out = np.zeros((T, H), dtype=np.float32)
    o = 0
    for e in range(NUM_EXPERTS):
        n = seg_idx[e].size
        if n == 0:
            continue
        Y = ysum[:, o : o + n].T + bdown[e]
        out[seg_idx[e]] += seg_wts[e][:, None].astype(np.float32) * Y
        o += n
    return out.reshape(orig_shape)


# revision 28
# speedup vs baseline: 1.2840x; 1.0386x over previous
"""MoE BERT block kernel for 8 Trainium2 NeuronCores.

Strategy: tensor-parallel over the expert FFN's INTER dimension. The router
(gate matmul + softmax + top-2) is a ~134 MFLOP computation done on the host
in float64 while packing the inputs; tokens are gathered into per-expert
segments on the host. Every core receives ALL 16384 token-expert pairs but
only a 512-wide slice of each expert's inter dimension:

    core c:  h_c   = gelu(Wup[e][c*512:(c+1)*512] @ x + bup_slice)   per token
             y_c   = Wdown[e][:, c*512:(c+1)*512] @ h_c              (partial)

gelu is elementwise in the inter dim, so y = sum_c y_c exactly. The host sums
the 8 f16 partials (float32 accumulate), adds bdown, and scatter-adds w * y.

Why this beats expert-parallel: per-core work is 64 matmul columns x 16384
tokens = 1.048M PE cycles regardless of the router outcome — perfect load
balance with zero token padding (expert-parallel pays for the heaviest
expert's 2161 tokens = 1.107M cycles). SBUF weight footprint is unchanged
(1/8 of every expert = 16.8 MB bf16).

DMA routing (descriptor issue costs ~0.6us of sequencer time per dma_start,
so the startup-critical stream must be few, large transfers in arrival
order): the sync (SP) ring carries tile 0's x per-ko chunks interleaved with
expert 0's up-weight io-chunks, expert 0's down weights, then the x stream
(prefetch distance 2) and per-tile y writebacks. The otherwise-idle GpSimd
software-DGE ring streams experts 1..7's weights as one ~1MB DMA per tensor,
far ahead of use. The scalar ring only runs GELUs.
"""

import os

os.environ.setdefault("MYCRO_LOCAL_CACHE", "1")

import numpy as np
import ml_dtypes

import concourse.bass as bass
import concourse.bacc as bacc
import concourse.mybir as mybir
import concourse.tile as tile
from concourse.bass_utils import run_bass_kernel_spmd

NUM_EXPERTS = 8
TOP_K = 2
H = 1024
I = 4096
P = 128
NCORES = 8
SLICE = I // NCORES  # 512 inter rows per core
KO = H // P  # 8 contraction tiles for the up matmul
IOL = SLICE // P  # 4 local inter tiles (psum partitions up / contraction down)
HO = H // P  # 8 output tiles for the down matmul
NMAX = 512  # max token tile (psum bank holds 512 f32)

BF16 = mybir.dt.bfloat16
F16 = mybir.dt.float16
F32 = mybir.dt.float32

_programs = {}  # schedule tuple -> compiled Bacc
last_results = None  # BassKernelResults of the most recent run (for profiling)


def _token_tiles(n):
    """Split n tokens into tiles: first tile NMAX, rest spread evenly in
    (128, 512]. The big first tile gives the DMA stream time to fill the
    pipeline before the next expert's weights are needed."""
    if n == 0:
        return []
    if n <= NMAX:
        return [n]
    k = -(-n // NMAX)  # ceil
    if n - NMAX <= (k - 1) * NMAX:
        rest = n - NMAX
        kk = k - 1
        base, rem = divmod(rest, kk)
        return [NMAX] + [base + 1] * rem + [base] * (kk - rem)
    base, rem = divmod(n, k)
    return [base + 1] * rem + [base] * (k - rem)


def _build_program(schedule):
    """schedule: tuple of (expert, ntok) tiles, concatenated token order."""
    TT = sum(nt for _, nt in schedule)
    nc = bacc.Bacc("TRN2", target_bir_lowering=False)

    # wup column order is io-major within an expert: [e][io][ko][128] so the
    # first io group's weights are one small contiguous chunk at startup.
    xt = nc.dram_tensor("xt", [P, KO * TT], BF16, kind="ExternalInput")
    wup = nc.dram_tensor("wup", [P, NUM_EXPERTS * IOL * KO * P], BF16, kind="ExternalInput")
    wdn = nc.dram_tensor("wdn", [P, NUM_EXPERTS * IOL * H], BF16, kind="ExternalInput")
    bup = nc.dram_tensor("bup", [P, NUM_EXPERTS * IOL], F32, kind="ExternalInput")
    yt = nc.dram_tensor("yt", [H, TT], F16, kind="ExternalOutput")

    experts_in_order = []
    for e, _ in schedule:
        if not experts_in_order or experts_in_order[-1] != e:
            experts_in_order.append(e)
    e0 = experts_in_order[0]

    with tile.TileContext(nc) as tc:
        with (
            tc.tile_pool(name="weights", bufs=1) as wpool,
            tc.tile_pool(name="xin", bufs=5) as xpool,
            tc.tile_pool(name="hmid", bufs=2) as hpool,
            tc.tile_pool(name="yout", bufs=20) as ypool,
            tc.tile_pool(name="psum_up", bufs=3, space="PSUM") as pu,
            tc.tile_pool(name="psum_dn", bufs=5, space="PSUM") as pd,
        ):
            yt_r = yt.ap().rearrange("(ho p) t -> p ho t", p=P)
            xt_ap = xt.ap()
            wup_ap = wup.ap()
            wdn_ap = wdn.ap()

            wup_sb = wpool.tile([P, NUM_EXPERTS, IOL * KO * P], BF16, tag="wup")
            wdn_sb = wpool.tile([P, NUM_EXPERTS, IOL * H], BF16, tag="wdn")
            bup_sb = wpool.tile([P, NUM_EXPERTS * IOL], F32, tag="bup")

            # Warmup operand tile. The warmup matmuls accumulate into a
            # throwaway psum group that is never read (its bank is later
            # reset by a real start=True group), so the values don't matter;
            # the memset only exists because the framework requires every
            # read tile to have a writer.
            xw_sb = wpool.tile([P, NMAX], BF16, tag="warmx")
            nc.vector.memset(xw_sb[:], 0.0)

            # --- Startup stream on the sync ring, in arrival-need order:
            # bup (tiny, needed by the first GELU), then tile 0's x per-ko
            # chunks interleaved with expert 0's up-weight io-chunks (the
            # first real matmul needs only x0[ko0] + wup[e0,io0]), then
            # expert 0's down weights, then x1/x2 (prefetch distance 2).
            _, n0 = schedule[0]
            x0_sb = xpool.tile([P, KO, NMAX], BF16, tag="x")
            x0_r = xt_ap[:, 0 : KO * n0].rearrange("p (ko t) -> p ko t", ko=KO)
            for ko in range(KO):
                nc.sync.dma_start(x0_sb[:, ko, :n0], x0_r[:, ko])
                if ko < IOL:
                    io = ko
                    col = (e0 * IOL + io) * KO * P
                    nc.sync.dma_start(
                        wup_sb[:, e0, io * KO * P : (io + 1) * KO * P],
                        wup_ap[:, col : col + KO * P],
                    )
            nc.sync.dma_start(bup_sb[:], bup.ap())
            nc.sync.dma_start(
                wdn_sb[:, e0], wdn_ap[:, e0 * IOL * H : (e0 + 1) * IOL * H]
            )

            # Experts 1..7 weights: one ~1MB DMA per tensor on the sync ring.
            # The tile scheduler orders each queue by dependency readiness,
            # so dep-free weight DMAs would all hoist into the startup window
            # and starve tile 0's critical transfers (that cost v2 ~35us of
            # PE gaps). tile_wait_until stamps them with staggered modeled
            # times instead: each expert streams during the previous expert's
            # compute, ~3 tiles before first use.
            for k, en in enumerate(experts_in_order[1:]):
                with tc.tile_wait_until(0.015 + 0.045 * k):
                    nc.sync.dma_start(
                        wup_sb[:, en, :],
                        wup_ap[:, en * IOL * KO * P : (en + 1) * IOL * KO * P],
                    )
                    nc.sync.dma_start(
                        wdn_sb[:, en, :],
                        wdn_ap[:, en * IOL * H : (en + 1) * IOL * H],
                    )

            # Warmup chain: 12 garbage matmuls bridge the PE from engine-up
            # to tile 0's operand arrival; 6 more keep-alives are interleaved
            # between tile 0 io0's arrival-paced ko steps (below).
            NWARM, NKEEP = 6, 6
            ps_trash = pd.tile([P, NMAX], F32, tag="pd", name="trash")
            warm_i = [0]

            def warm_mm():
                nc.tensor.matmul(
                    ps_trash[:, :NMAX], lhsT=xw_sb[:, :P], rhs=xw_sb[:, :NMAX],
                    start=(warm_i[0] == 0),
                    stop=(warm_i[0] == NWARM + NKEEP - 1),
                )
                warm_i[0] += 1

            for _ in range(NWARM):
                warm_mm()

            offsets = []
            o = 0
            for _, nt in schedule:
                offsets.append(o)
                o += nt

            def prefetch_x(tp):
                npre = schedule[tp][1]
                opre = offsets[tp]
                xp = xpool.tile([P, KO, NMAX], BF16, tag="x")
                nc.sync.dma_start(
                    xp[:, :, :npre],
                    xt_ap[:, KO * opre : KO * (opre + npre)].rearrange(
                        "p (ko t) -> p ko t", ko=KO
                    ),
                )
                return xp

            XPRE = 4  # x prefetch distance (xpool bufs - 1)
            x_tiles = {0: x0_sb}
            for tp in range(1, XPRE + 1):
                if tp < len(schedule):
                    x_tiles[tp] = prefetch_x(tp)

            off = 0
            for t, (e, ntok) in enumerate(schedule):
                x_sb = x_tiles.pop(t)
                if t >= 1 and t + XPRE < len(schedule):
                    x_tiles[t + XPRE] = prefetch_x(t + XPRE)

                # --- Up-projection + exact (erf) GELU: h tile [512, ntok].
                # io-major (contraction inner): each group's GELU drains while
                # the next group accumulates — no end-of-tile burst.
                h_sb = hpool.tile([P, IOL, NMAX], BF16, tag="h")
                for io in range(IOL):
                    ps = pu.tile([P, NMAX], F32, tag="pu", name=f"pu{io}")
                    for ko in range(KO):
                        col = io * KO * P + ko * P
                        nc.tensor.matmul(
                            ps[:, :ntok],
                            lhsT=wup_sb[:, e, col : col + P],
                            rhs=x_sb[:, ko, :ntok],
                            start=(ko == 0),
                            stop=(ko == KO - 1),
                        )
                        if t == 0 and io == 0 and ko < NKEEP:
                            # Keep-alive between arrival-paced first ko steps.
                            warm_mm()
                    nc.scalar.activation(
                        h_sb[:, io, :ntok], ps[:, :ntok],
                        mybir.ActivationFunctionType.Gelu,
                        bias=bup_sb[:, e * IOL + io : e * IOL + io + 1],
                        scale=1.0,
                    )
                # --- Down-projection partial: y tile [1024, ntok] f16.
                # ho-major, contraction (4 io steps) inner. The psum->f16 cast
                # (DVE) overlaps the next ho's matmuls. y buffers and DMAs are
                # per-ho: under an HBM-bandwidth drought (cotenant noise) the
                # 20-slot rotation gives ~2.5 tiles of writeback slack before
                # the psum-bank chain would stall the PE.
                for ho in range(HO):
                    ps = pd.tile([P, NMAX], F32, tag="pd", name=f"pd{ho % 4}")
                    for io in range(IOL):
                        col = io * H + ho * P
                        nc.tensor.matmul(
                            ps[:, :ntok],
                            lhsT=wdn_sb[:, e, col : col + P],
                            rhs=h_sb[:, io, :ntok],
                            start=(io == 0),
                            stop=(io == IOL - 1),
                        )
                    y_sb = ypool.tile([P, NMAX], F16, tag="y")
                    nc.vector.tensor_scalar_add(
                        y_sb[:, :ntok], ps[:, :ntok], 0.0
                    )
                    nc.sync.dma_start(
                        yt_r[:, ho, off : off + ntok], y_sb[:, :ntok]
                    )
                off += ntok

    nc.compile()
    return nc


def _get_program(schedule):
    key = tuple(schedule)
    if key not in _programs:
        _programs[key] = _build_program(key)
    return _programs[key]


def _route(X64, Wg64):
    """Replicates the reference router: softmax over gate logits, top-2."""
    T = X64.shape[0]
    logits = X64 @ Wg64.T  # [T, E]
    logits -= logits.max(axis=-1, keepdims=True)
    p = np.exp(logits)
    p /= p.sum(axis=-1, keepdims=True)
    i1 = np.argmax(p, axis=-1)
    rows = np.arange(T)
    w1 = p[rows, i1]
    p2 = p.copy()
    p2[rows, i1] = -1.0
    i2 = np.argmax(p2, axis=-1)
    w2 = p[rows, i2]
    return i1, w1, i2, w2


def _pack_core(c, Wup16, Wdn16, bup):
    rows = slice(c * SLICE, (c + 1) * SLICE)
    wup_c = np.empty((P, NUM_EXPERTS * IOL * KO * P), dtype=ml_dtypes.bfloat16)
    wdn_c = np.empty((P, NUM_EXPERTS * IOL * H), dtype=ml_dtypes.bfloat16)
    bup_c = np.empty((P, NUM_EXPERTS * IOL), dtype=np.float32)
    for e in range(NUM_EXPERTS):
        # up lhsT: [k partition, (io, ko, 128 io-rows)] io-major chunks
        w = Wup16[e][rows, :].T.reshape(KO, P, IOL, P)  # [ko, kp, io, m]
        w = w.transpose(1, 2, 0, 3)  # [kp, io, ko, m]
        wup_c[:, e * IOL * KO * P : (e + 1) * IOL * KO * P] = w.reshape(P, -1)
        # down lhsT: [local-inter partition, (io, H cols)]
        d = Wdn16[e][:, rows].T.reshape(IOL, P, H).transpose(1, 0, 2)
        wdn_c[:, e * IOL * H : (e + 1) * IOL * H] = d.reshape(P, -1)
        bup_c[:, e * IOL : (e + 1) * IOL] = bup[e][rows].reshape(IOL, P).T
    return (
        np.ascontiguousarray(wup_c),
        np.ascontiguousarray(wdn_c),
        bup_c,
    )


def kernel(hidden_states, Wg, Wup, bup, Wdown, bdown):
    global last_results
    hidden_states = np.asarray(hidden_states)
    orig_shape = hidden_states.shape
    X = np.ascontiguousarray(hidden_states, dtype=np.float32).reshape(-1, H)
    T = X.shape[0]
    Wg = np.asarray(Wg, dtype=np.float32)
    Wup = np.asarray(Wup, dtype=np.float32)
    bup = np.asarray(bup, dtype=np.float32)
    Wdown = np.asarray(Wdown, dtype=np.float32)
    bdown = np.asarray(bdown, dtype=np.float32)

    # --- Router on host (float64 for a faithful top-2 ordering) ---
    i1, w1, i2, w2 = _route(X.astype(np.float64), Wg.astype(np.float64))

    # --- Dispatch: gather tokens into per-expert segments (i1 then i2) ---
    seg_idx, seg_wts = [], []
    schedule = []
    for e in range(NUM_EXPERTS):
        sel1 = np.nonzero(i1 == e)[0]
        sel2 = np.nonzero(i2 == e)[0]
        idx = np.concatenate([sel1, sel2])
        wts = np.concatenate([w1[sel1], w2[sel2]])
        seg_idx.append(idx)
        seg_wts.append(wts)
        schedule.extend((e, nt) for nt in _token_tiles(idx.size))
    schedule = tuple(schedule)
    idx_all = np.concatenate(seg_idx)

    # --- Pack device inputs ---
    Xb = X.astype(ml_dtypes.bfloat16)
    Xsel = Xb[idx_all]  # [TT, H]
    blocks = []
    o = 0
    for _, nt in schedule:
        blk = Xsel[o : o + nt].T.reshape(KO, P, nt)  # [KO, P, nt]
        blocks.append(blk.transpose(1, 0, 2).reshape(P, -1))
        o += nt
    xt_dev = np.ascontiguousarray(np.concatenate(blocks, axis=1))

    Wup16 = Wup.astype(ml_dtypes.bfloat16)
    Wdn16 = Wdown.astype(ml_dtypes.bfloat16)
    in_maps = []
    for c in range(NCORES):
        wup_c, wdn_c, bup_c = _pack_core(c, Wup16, Wdn16, bup)
        in_maps.append({"xt": xt_dev, "wup": wup_c, "wdn": wdn_c, "bup": bup_c})

    # --- Run the Bass kernel on all 8 cores ---
    nc = _get_program(schedule)
    last_results = run_bass_kernel_spmd(nc, in_maps, core_ids=list(range(NCORES)))

    # --- Combine: sum the 8 inter-slice partials, add bdown, scatter w * y ---
    ysum = np.zeros((H, sum(nt for _, nt in schedule)), dtype=np.float32)
    for c in range(NCORES):
        ysum += np.asarray(last_results.results[c]["yt"]).astype(np.float32)

    out = np.zeros((T, H), dtype=np.float32)
    o = 0
    for e in range(NUM_EXPERTS):
        n = seg_idx[e].size
        if n == 0:
            continue
        Y = ysum[:, o : o + n].T + bdown[e]
        out[seg_idx[e]] += seg_wts[e][:, None].astype(np.float32) * Y
        o += n
    return out.reshape(orig_shape)


# revision 30
# speedup vs baseline: 1.2844x; 1.0003x over previous
"""MoE BERT block kernel for 8 Trainium2 NeuronCores.

Strategy: tensor-parallel over the expert FFN's INTER dimension. The router
(gate matmul + softmax + top-2) is a ~134 MFLOP computation done on the host
in float64 while packing the inputs; tokens are gathered into per-expert
segments on the host. Every core receives ALL 16384 token-expert pairs but
only a 512-wide slice of each expert's inter dimension:

    core c:  h_c   = gelu(Wup[e][c*512:(c+1)*512] @ x + bup_slice)   per token
             y_c   = Wdown[e][:, c*512:(c+1)*512] @ h_c              (partial)

gelu is elementwise in the inter dim, so y = sum_c y_c exactly. The host sums
the 8 f16 partials (float32 accumulate), adds bdown, and scatter-adds w * y.

Why this beats expert-parallel: per-core work is 64 matmul columns x 16384
tokens = 1.048M PE cycles regardless of the router outcome — perfect load
balance with zero token padding (expert-parallel pays for the heaviest
expert's 2161 tokens = 1.107M cycles). SBUF weight footprint is unchanged
(1/8 of every expert = 16.8 MB bf16).

DMA routing (descriptor issue costs ~0.6us of sequencer time per dma_start,
so the startup-critical stream must be few, large transfers in arrival
order): the sync (SP) ring carries tile 0's x per-ko chunks interleaved with
expert 0's up-weight io-chunks, expert 0's down weights, then the x stream
(prefetch distance 2) and per-tile y writebacks. The otherwise-idle GpSimd
software-DGE ring streams experts 1..7's weights as one ~1MB DMA per tensor,
far ahead of use. The scalar ring only runs GELUs.
"""

import os

os.environ.setdefault("MYCRO_LOCAL_CACHE", "1")

import numpy as np
import ml_dtypes

import concourse.bass as bass
import concourse.bacc as bacc
import concourse.mybir as mybir
import concourse.tile as tile
from concourse.bass_utils import run_bass_kernel_spmd

NUM_EXPERTS = 8
TOP_K = 2
H = 1024
I = 4096
P = 128
NCORES = 8
SLICE = I // NCORES  # 512 inter rows per core
KO = H // P  # 8 contraction tiles for the up matmul
IOL = SLICE // P  # 4 local inter tiles (psum partitions up / contraction down)
HO = H // P  # 8 output tiles for the down matmul
NMAX = 512  # max token tile (psum bank holds 512 f32)

BF16 = mybir.dt.bfloat16
F16 = mybir.dt.float16
F32 = mybir.dt.float32

_programs = {}  # schedule tuple -> compiled Bacc
last_results = None  # BassKernelResults of the most recent run (for profiling)


def _token_tiles(n):
    """Split n tokens into tiles: first tile NMAX, rest spread evenly in
    (128, 512]. The big first tile gives the DMA stream time to fill the
    pipeline before the next expert's weights are needed."""
    if n == 0:
        return []
    if n <= NMAX:
        return [n]
    k = -(-n // NMAX)  # ceil
    if n - NMAX <= (k - 1) * NMAX:
        rest = n - NMAX
        kk = k - 1
        base, rem = divmod(rest, kk)
        return [NMAX] + [base + 1] * rem + [base] * (kk - rem)
    base, rem = divmod(n, k)
    return [base + 1] * rem + [base] * (k - rem)


def _build_program(schedule):
    """schedule: tuple of (expert, ntok) tiles, concatenated token order."""
    TT = sum(nt for _, nt in schedule)
    nc = bacc.Bacc("TRN2", target_bir_lowering=False)

    # wup column order is io-major within an expert: [e][io][ko][128] so the
    # first io group's weights are one small contiguous chunk at startup.
    xt = nc.dram_tensor("xt", [P, KO * TT], BF16, kind="ExternalInput")
    wup = nc.dram_tensor("wup", [P, NUM_EXPERTS * IOL * KO * P], BF16, kind="ExternalInput")
    wdn = nc.dram_tensor("wdn", [P, NUM_EXPERTS * IOL * H], BF16, kind="ExternalInput")
    bup = nc.dram_tensor("bup", [P, NUM_EXPERTS * IOL], F32, kind="ExternalInput")
    yt = nc.dram_tensor("yt", [H, TT], F16, kind="ExternalOutput")

    experts_in_order = []
    for e, _ in schedule:
        if not experts_in_order or experts_in_order[-1] != e:
            experts_in_order.append(e)
    e0 = experts_in_order[0]

    with tile.TileContext(nc) as tc:
        with (
            tc.tile_pool(name="weights", bufs=1) as wpool,
            tc.tile_pool(name="xin", bufs=5) as xpool,
            tc.tile_pool(name="hmid", bufs=2) as hpool,
            tc.tile_pool(name="yout", bufs=20) as ypool,
            tc.tile_pool(name="psum_up", bufs=3, space="PSUM") as pu,
            tc.tile_pool(name="psum_dn", bufs=5, space="PSUM") as pd,
        ):
            yt_r = yt.ap().rearrange("(ho p) t -> p ho t", p=P)
            xt_ap = xt.ap()
            wup_ap = wup.ap()
            wdn_ap = wdn.ap()

            wup_sb = wpool.tile([P, NUM_EXPERTS, IOL * KO * P], BF16, tag="wup")
            wdn_sb = wpool.tile([P, NUM_EXPERTS, IOL * H], BF16, tag="wdn")
            bup_sb = wpool.tile([P, NUM_EXPERTS * IOL], F32, tag="bup")

            # Warmup operand tile. The warmup matmuls accumulate into a
            # throwaway psum group that is never read (its bank is later
            # reset by a real start=True group), so the values don't matter;
            # the memset only exists because the framework requires every
            # read tile to have a writer.
            xw_sb = wpool.tile([P, NMAX], BF16, tag="warmx")
            nc.vector.memset(xw_sb[:], 0.0)

            # --- Startup stream on the sync ring, in arrival-need order:
            # bup (tiny, needed by the first GELU), then tile 0's x per-ko
            # chunks interleaved with expert 0's up-weight io-chunks (the
            # first real matmul needs only x0[ko0] + wup[e0,io0]), then
            # expert 0's down weights, then x1/x2 (prefetch distance 2).
            _, n0 = schedule[0]
            x0_sb = xpool.tile([P, KO, NMAX], BF16, tag="x")
            x0_r = xt_ap[:, 0 : KO * n0].rearrange("p (ko t) -> p ko t", ko=KO)
            for ko in range(KO):
                nc.sync.dma_start(x0_sb[:, ko, :n0], x0_r[:, ko])
                if ko < IOL:
                    io = ko
                    col = (e0 * IOL + io) * KO * P
                    nc.sync.dma_start(
                        wup_sb[:, e0, io * KO * P : (io + 1) * KO * P],
                        wup_ap[:, col : col + KO * P],
                    )
            nc.sync.dma_start(bup_sb[:], bup.ap())
            nc.sync.dma_start(
                wdn_sb[:, e0], wdn_ap[:, e0 * IOL * H : (e0 + 1) * IOL * H]
            )

            # Experts 1..7 weights: one ~1MB DMA per tensor on the sync ring.
            # The tile scheduler orders each queue by dependency readiness,
            # so dep-free weight DMAs would all hoist into the startup window
            # and starve tile 0's critical transfers (that cost v2 ~35us of
            # PE gaps). tile_wait_until stamps them with staggered modeled
            # times instead: each expert streams during the previous expert's
            # compute, ~3 tiles before first use.
            for k, en in enumerate(experts_in_order[1:]):
                with tc.tile_wait_until(0.015 + 0.045 * k):
                    nc.sync.dma_start(
                        wup_sb[:, en, :],
                        wup_ap[:, en * IOL * KO * P : (en + 1) * IOL * KO * P],
                    )
                    nc.sync.dma_start(
                        wdn_sb[:, en, :],
                        wdn_ap[:, en * IOL * H : (en + 1) * IOL * H],
                    )

            # Warmup chain: 12 garbage matmuls bridge the PE from engine-up
            # to tile 0's operand arrival; 6 more keep-alives are interleaved
            # between tile 0 io0's arrival-paced ko steps (below).
            NWARM, NKEEP = 6, 10
            ps_trash = pd.tile([P, NMAX], F32, tag="pd", name="trash")
            warm_i = [0]

            def warm_mm():
                nc.tensor.matmul(
                    ps_trash[:, :NMAX], lhsT=xw_sb[:, :P], rhs=xw_sb[:, :NMAX],
                    start=(warm_i[0] == 0),
                    stop=(warm_i[0] == NWARM + NKEEP - 1),
                )
                warm_i[0] += 1

            for _ in range(NWARM):
                warm_mm()

            offsets = []
            o = 0
            for _, nt in schedule:
                offsets.append(o)
                o += nt

            def prefetch_x(tp):
                npre = schedule[tp][1]
                opre = offsets[tp]
                xp = xpool.tile([P, KO, NMAX], BF16, tag="x")
                nc.sync.dma_start(
                    xp[:, :, :npre],
                    xt_ap[:, KO * opre : KO * (opre + npre)].rearrange(
                        "p (ko t) -> p ko t", ko=KO
                    ),
                )
                return xp

            XPRE = 4  # x prefetch distance (xpool bufs - 1)
            x_tiles = {0: x0_sb}
            for tp in range(1, XPRE + 1):
                if tp < len(schedule):
                    x_tiles[tp] = prefetch_x(tp)

            off = 0
            for t, (e, ntok) in enumerate(schedule):
                x_sb = x_tiles.pop(t)
                if t >= 1 and t + XPRE < len(schedule):
                    x_tiles[t + XPRE] = prefetch_x(t + XPRE)

                # --- Up-projection + exact (erf) GELU: h tile [512, ntok].
                # io-major (contraction inner): each group's GELU drains while
                # the next group accumulates — no end-of-tile burst.
                h_sb = hpool.tile([P, IOL, NMAX], BF16, tag="h")
                for io in range(IOL):
                    ps = pu.tile([P, NMAX], F32, tag="pu", name=f"pu{io}")
                    for ko in range(KO):
                        col = io * KO * P + ko * P
                        nc.tensor.matmul(
                            ps[:, :ntok],
                            lhsT=wup_sb[:, e, col : col + P],
                            rhs=x_sb[:, ko, :ntok],
                            start=(ko == 0),
                            stop=(ko == KO - 1),
                        )
                        if t == 0 and (
                            (io == 0 and ko < 7) or (io == 1 and 4 <= ko < 7)
                        ):
                            # Keep-alive between arrival-paced early ko steps
                            # (the 3.2MB x0+wup_e0 stream lands ~13.8us, pacing
                            # io0's chain and io1's tail).
                            warm_mm()
                    nc.scalar.activation(
                        h_sb[:, io, :ntok], ps[:, :ntok],
                        mybir.ActivationFunctionType.Gelu,
                        bias=bup_sb[:, e * IOL + io : e * IOL + io + 1],
                        scale=1.0,
                    )
                # --- Down-projection partial: y tile [1024, ntok] f16.
                # ho-major, contraction (4 io steps) inner. The psum->f16 cast
                # (DVE) overlaps the next ho's matmuls. y buffers and DMAs are
                # per-ho: under an HBM-bandwidth drought (cotenant noise) the
                # 20-slot rotation gives ~2.5 tiles of writeback slack before
                # the psum-bank chain would stall the PE.
                for ho in range(HO):
                    ps = pd.tile([P, NMAX], F32, tag="pd", name=f"pd{ho % 4}")
                    for io in range(IOL):
                        col = io * H + ho * P
                        nc.tensor.matmul(
                            ps[:, :ntok],
                            lhsT=wdn_sb[:, e, col : col + P],
                            rhs=h_sb[:, io, :ntok],
                            start=(io == 0),
                            stop=(io == IOL - 1),
                        )
                    y_sb = ypool.tile([P, NMAX], F16, tag="y")
                    nc.vector.tensor_scalar_add(
                        y_sb[:, :ntok], ps[:, :ntok], 0.0
                    )
                    nc.sync.dma_start(
                        yt_r[:, ho, off : off + ntok], y_sb[:, :ntok]
                    )
                off += ntok

    nc.compile()
    return nc


def _get_program(schedule):
    key = tuple(schedule)
    if key not in _programs:
        _programs[key] = _build_program(key)
    return _programs[key]


def _route(X64, Wg64):
    """Replicates the reference router: softmax over gate logits, top-2."""
    T = X64.shape[0]
    logits = X64 @ Wg64.T  # [T, E]
    logits -= logits.max(axis=-1, keepdims=True)
    p = np.exp(logits)
    p /= p.sum(axis=-1, keepdims=True)
    i1 = np.argmax(p, axis=-1)
    rows = np.arange(T)
    w1 = p[rows, i1]
    p2 = p.copy()
    p2[rows, i1] = -1.0
    i2 = np.argmax(p2, axis=-1)
    w2 = p[rows, i2]
    return i1, w1, i2, w2


def _pack_core(c, Wup16, Wdn16, bup):
    rows = slice(c * SLICE, (c + 1) * SLICE)
    wup_c = np.empty((P, NUM_EXPERTS * IOL * KO * P), dtype=ml_dtypes.bfloat16)
    wdn_c = np.empty((P, NUM_EXPERTS * IOL * H), dtype=ml_dtypes.bfloat16)
    bup_c = np.empty((P, NUM_EXPERTS * IOL), dtype=np.float32)
    for e in range(NUM_EXPERTS):
        # up lhsT: [k partition, (io, ko, 128 io-rows)] io-major chunks
        w = Wup16[e][rows, :].T.reshape(KO, P, IOL, P)  # [ko, kp, io, m]
        w = w.transpose(1, 2, 0, 3)  # [kp, io, ko, m]
        wup_c[:, e * IOL * KO * P : (e + 1) * IOL * KO * P] = w.reshape(P, -1)
        # down lhsT: [local-inter partition, (io, H cols)]
        d = Wdn16[e][:, rows].T.reshape(IOL, P, H).transpose(1, 0, 2)
        wdn_c[:, e * IOL * H : (e + 1) * IOL * H] = d.reshape(P, -1)
        bup_c[:, e * IOL : (e + 1) * IOL] = bup[e][rows].reshape(IOL, P).T
    return (
        np.ascontiguousarray(wup_c),
        np.ascontiguousarray(wdn_c),
        bup_c,
    )


def kernel(hidden_states, Wg, Wup, bup, Wdown, bdown):
    global last_results
    hidden_states = np.asarray(hidden_states)
    orig_shape = hidden_states.shape
    X = np.ascontiguousarray(hidden_states, dtype=np.float32).reshape(-1, H)
    T = X.shape[0]
    Wg = np.asarray(Wg, dtype=np.float32)
    Wup = np.asarray(Wup, dtype=np.float32)
    bup = np.asarray(bup, dtype=np.float32)
    Wdown = np.asarray(Wdown, dtype=np.float32)
    bdown = np.asarray(bdown, dtype=np.float32)

    # --- Router on host (float64 for a faithful top-2 ordering) ---
    i1, w1, i2, w2 = _route(X.astype(np.float64), Wg.astype(np.float64))

    # --- Dispatch: gather tokens into per-expert segments (i1 then i2) ---
    seg_idx, seg_wts = [], []
    schedule = []
    for e in range(NUM_EXPERTS):
        sel1 = np.nonzero(i1 == e)[0]
        sel2 = np.nonzero(i2 == e)[0]
        idx = np.concatenate([sel1, sel2])
        wts = np.concatenate([w1[sel1], w2[sel2]])
        seg_idx.append(idx)
        seg_wts.append(wts)
        schedule.extend((e, nt) for nt in _token_tiles(idx.size))
    schedule = tuple(schedule)
    idx_all = np.concatenate(seg_idx)

    # --- Pack device inputs ---
    Xb = X.astype(ml_dtypes.bfloat16)
    Xsel = Xb[idx_all]  # [TT, H]
    blocks = []
    o = 0
    for _, nt in schedule:
        blk = Xsel[o : o + nt].T.reshape(KO, P, nt)  # [KO, P, nt]
        blocks.append(blk.transpose(1, 0, 2).reshape(P, -1))
        o += nt
    xt_dev = np.ascontiguousarray(np.concatenate(blocks, axis=1))

    Wup16 = Wup.astype(ml_dtypes.bfloat16)
    Wdn16 = Wdown.astype(ml_dtypes.bfloat16)
    in_maps = []
    for c in range(NCORES):
        wup_c, wdn_c, bup_c = _pack_core(c, Wup16, Wdn16, bup)
        in_maps.append({"xt": xt_dev, "wup": wup_c, "wdn": wdn_c, "bup": bup_c})

    # --- Run the Bass kernel on all 8 cores ---
    nc = _get_program(schedule)
    last_results = run_bass_kernel_spmd(nc, in_maps, core_ids=list(range(NCORES)))

    # --- Combine: sum the 8 inter-slice partials, add bdown, scatter w * y ---
    ysum = np.zeros((H, sum(nt for _, nt in schedule)), dtype=np.float32)
    for c in range(NCORES):
        ysum += np.asarray(last_results.results[c]["yt"]).astype(np.float32)

    out = np.zeros((T, H), dtype=np.float32)
    o = 0
    for e in range(NUM_EXPERTS):
        n = seg_idx[e].size
        if n == 0:
            continue
        Y = ysum[:, o : o + n].T + bdown[e]
        out[seg_idx[e]] += seg_wts[e][:, None].astype(np.float32) * Y
        o += n
    return out.reshape(orig_shape)
